# revision 15
# baseline (speedup 1.0000x reference)
"""MACE MessagePassingConvolution on 8 Trainium2 NeuronCores.

Strategy (graph/data parallel over edges, sorted by receiver):
  - Host: sort edges by receiver, split into 8 contiguous chunks at node
    boundaries.  Within a core, group edges into variable-width node
    "windows" (<=128 nodes and <=T_TILES*128 edges each), padded to a
    static grid of W windows x T_TILES tiles of 128 edges.  node_feats
    columns are pre-shuffled to [s | vx | vy | vz] so vector components are
    i-major on device; the output's vector columns are un-permuted on the
    host afterwards.
  - Device, per 128-edge tile: dma_gather of sender features (fp16),
    radial-MLP gates (PE matmuls, edges on the free dim; the orientation
    flips at the last layer by making h3^T the stationary operand), message
    construction with chunk-batched fp16 vector ops laid out to keep the
    DVE 2x perf mode (long contiguous inner runs), and a one-hot selection
    matmul that scatter-accumulates the 1408-wide fp16 messages into fp32
    PSUM window accumulators.
  - Each window drains PSUM -> SBUF -> DRAM rows at static offsets; the
    host reassembles per-core slabs into the full [N, 1408] output.
"""

import os
import sys

for _p in ("/opt/trn_rl_repo", "/root/.axon_site/_ro/trn_rl_repo"):
    if os.path.isdir(_p) and _p not in sys.path:
        sys.path.insert(0, _p)

import numpy as np

# ---------------------------------------------------------------- constants
C = 128
N_NODES = 10000
N_EDGES = 100000
R_DIM = 8
NCORES = 8
P = 128

T_TILES = 8                 # 128-edge tiles per window (must be mult of 4)
EDGE_CAP = T_TILES * P      # max edges per window
CHUNK = 512                 # edges per radial-MLP chunk (4 tiles)

AVG_NUM_NEIGHBORS = 10.0
SH_L2 = np.sqrt(7.5)
CG_121 = np.sqrt(0.4)
# CG_011*SH_L1 == 1 and CG_110*SH_L1 == 1 exactly, so tp0/tp1 need no
# extra scale; tp2's scale is CG_121*SH_L2 == sqrt(3).
G4_SCALE = CG_121 * SH_L2

_CACHE: dict = {}
_LAST_IN_MAPS = None


def _install_ntff_hook():
    """Make run_bass_kernel_spmd(trace=True) usable under axon: the agent
    image's antenv lacks axon_hooks, so recreate the registry and install
    the ctypes NTFF hook. Harmless if profiling is never requested."""
    import types

    if "antenv.axon_hooks" in sys.modules:
        return
    mod = types.ModuleType("antenv.axon_hooks")
    _h = [None]
    mod.set_axon_ntff_profile_hook = lambda h: _h.__setitem__(0, h)
    mod.get_axon_ntff_profile_hook = lambda: _h[0]
    sys.modules["antenv.axon_hooks"] = mod
    try:
        import antenv

        antenv.axon_hooks = mod
    except ImportError:
        pass
    try:
        from trn_agent_boot.trn_boot import _ntff_profile_via_ctypes

        h = _ntff_profile_via_ctypes("/opt/axon/libaxon_pjrt.so")
        if h is not None:
            mod.set_axon_ntff_profile_hook(h)
    except Exception:
        pass


# ---------------------------------------------------------------- host prep
def _make_windows(senders, receivers, n_nodes):
    """Sort edges by receiver, shard into NCORES chunks at node boundaries,
    then split each core's node range into windows of <=128 nodes and
    <=EDGE_CAP edges."""
    E = senders.shape[0]
    perm = np.argsort(receivers, kind="stable")
    recv_s = receivers[perm]

    splits = [0]
    for c in range(1, NCORES):
        t = (c * E) // NCORES
        while 0 < t < E and recv_s[t] == recv_s[t - 1]:
            t += 1
        splits.append(t)
    splits.append(E)

    deg = np.bincount(receivers, minlength=n_nodes)

    cores = []
    for c in range(NCORES):
        e0, e1 = splits[c], splits[c + 1]
        node_lo = 0 if c == 0 else (int(recv_s[e0]) if e0 < E else n_nodes)
        node_hi = int(recv_s[e1]) if e1 < E else n_nodes
        if c == NCORES - 1:
            node_hi = n_nodes
        if e0 == e1:
            node_lo = node_hi
        wins = []  # (node_start, node_len, edge_start, edge_count)
        n, e = node_lo, e0
        while n < node_hi:
            ns, ecnt = n, 0
            while n < node_hi and n - ns < P:
                d = int(deg[n])
                if ecnt + d > EDGE_CAP:
                    break
                ecnt += d
                n += 1
            assert n > ns, f"node {ns} degree {deg[ns]} exceeds window cap"
            wins.append((ns, n - ns, e, ecnt))
            e += ecnt
        assert e == e1, (c, e, e1)
        cores.append({"wins": wins})
    return perm, recv_s, cores


def _prep_core_arrays(W, wins, perm, recv_s, senders, vectors, radial):
    """Padded per-core device arrays for a static W x T_TILES grid."""
    f16 = np.float16
    L = W * EDGE_CAP
    snd = np.zeros(L, np.int16)
    rrv = np.full(L, -1.0, np.float32)
    vec = np.zeros((L, 3), np.float32)
    vec[:, 0] = 1.0  # pad vectors normalize safely
    rad = np.zeros((L, R_DIM), np.float32)
    for w, (ns, nl, es, ec) in enumerate(wins):
        o = w * EDGE_CAP
        idx = perm[es:es + ec]
        snd[o:o + ec] = senders[idx].astype(np.int16)
        rrv[o:o + ec] = (recv_s[es:es + ec] - ns).astype(np.float32)
        vec[o:o + ec] = vectors[idx]
        rad[o:o + ec] = radial[idx]
    # senders: dma_gather wrapped layout [W, 128, EDGE_CAP//16]
    # idx k of a window lives at [k%16, k//16], replicated to 128 partitions
    sndW = snd.reshape(W, EDGE_CAP // 16, 16).transpose(0, 2, 1)
    sndT = np.tile(sndW, (1, 8, 1)).copy()
    rrvT = rrv.reshape(W, T_TILES, P).transpose(0, 2, 1).astype(f16).copy()
    # vec: all windows in one [128, W*T*3] tensor (tile t of window w at
    # columns (w*T+t)*3 : +3)
    vecT = vec.reshape(W * T_TILES, P, 3).transpose(1, 0, 2).reshape(
        P, W * T_TILES * 3).copy()
    # radial: per 512-edge chunk transposed to [8, 512]
    radT = rad.reshape(L // CHUNK, CHUNK, R_DIM).transpose(0, 2, 1).astype(
        f16).copy()
    return sndT, rrvT, vecT, radT


# ---------------------------------------------------------------- bass build
def _build_module(W):
    import concourse.bass as bass  # noqa: F401
    import concourse.mybir as mybir
    import concourse.tile as tile
    from concourse import bacc
    from concourse.alu_op_type import AluOpType

    f32 = mybir.dt.float32
    f16 = mybir.dt.float16
    i16 = mybir.dt.int16
    AF = mybir.ActivationFunctionType
    X = mybir.AxisListType.X
    NT = W * T_TILES  # total tiles

    nc = bacc.Bacc("TRN2")

    # node_feats columns pre-shuffled on host to [s | vx | vy | vz]
    nf_d = nc.dram_tensor("node_feats", [N_NODES, 4 * C], f16,
                          kind="ExternalInput")
    w0_d = nc.dram_tensor("w0p", [R_DIM, 64], f16, kind="ExternalInput")
    w1_d = nc.dram_tensor("w1p", [64, 64], f16, kind="ExternalInput")
    w2_d = nc.dram_tensor("w2p", [64, 64], f16, kind="ExternalInput")
    w3a_d = nc.dram_tensor("w3a", [64, 512], f16, kind="ExternalInput")
    w3b_d = nc.dram_tensor("w3b", [64, 256], f16, kind="ExternalInput")
    iota_d = nc.dram_tensor("iota", [P, P], f16, kind="ExternalInput")
    snd_d = nc.dram_tensor("snd", [W, P, EDGE_CAP // 16], i16,
                           kind="ExternalInput")
    rrv_d = nc.dram_tensor("rrv", [W, P, T_TILES], f16, kind="ExternalInput")
    vec_d = nc.dram_tensor("vec", [P, NT * 3], f32, kind="ExternalInput")
    radT_d = nc.dram_tensor("radT", [W * EDGE_CAP // CHUNK, R_DIM, CHUNK],
                            f16, kind="ExternalInput")
    out_d = nc.dram_tensor("out", [W * P, 11 * C], f32, kind="ExternalOutput")

    with tile.TileContext(nc) as tc:
        with (
            tc.tile_pool(name="const", bufs=1) as cp,
            tc.tile_pool(name="win", bufs=3) as wp,
            tc.tile_pool(name="chunk", bufs=3) as kp,
            tc.tile_pool(name="tp", bufs=4) as tp,
            tc.tile_pool(name="pwin", bufs=2, space="PSUM") as pwin,
            tc.tile_pool(name="ph", bufs=1, space="PSUM") as ph,
        ):
            w0sb = cp.tile([R_DIM, 64], f16)
            nc.sync.dma_start(out=w0sb[:], in_=w0_d[:])
            w1sb = cp.tile([64, 64], f16)
            nc.sync.dma_start(out=w1sb[:], in_=w1_d[:])
            w2sb = cp.tile([64, 64], f16)
            nc.sync.dma_start(out=w2sb[:], in_=w2_d[:])
            w3asb = cp.tile([64, 512], f16)
            nc.sync.dma_start(out=w3asb[:], in_=w3a_d[:])
            w3bsb = cp.tile([64, 256], f16)
            nc.sync.dma_start(out=w3bsb[:], in_=w3b_d[:])
            iotasb = cp.tile([P, P], f16)
            nc.sync.dma_start(out=iotasb[:], in_=iota_d[:])

            # --- batched edge-vector normalization (one Sqrt table load) ---
            vecall = cp.tile([P, NT * 3], f32)
            nc.sync.dma_start(out=vecall[:], in_=vec_d[:])
            sqall = cp.tile([P, NT * 3], f32)
            nc.vector.tensor_tensor(sqall[:], vecall[:], vecall[:],
                                    op=AluOpType.mult)
            s1all = cp.tile([P, NT], f32)
            nc.vector.reduce_sum(
                s1all[:], sqall[:].rearrange("p (t i) -> p t i", i=3), axis=X)
            srall = cp.tile([P, NT], f32)
            nc.scalar.activation(srall[:], s1all[:], AF.Sqrt)
            rinvall = cp.tile([P, NT], f32)
            nc.vector.reciprocal(rinvall[:], srall[:])
            rnall = cp.tile([P, NT * 3], f16)
            nc.vector.tensor_tensor(
                rnall[:].rearrange("p (t i) -> p t i", i=3),
                vecall[:].rearrange("p (t i) -> p t i", i=3),
                rinvall[:].unsqueeze(2).to_broadcast([P, NT, 3]),
                op=AluOpType.mult)

            for w in range(W):
                sndw = wp.tile([P, EDGE_CAP // 16], i16, tag="sndw")
                nc.sync.dma_start(out=sndw[:], in_=snd_d[w])
                rrvw = wp.tile([P, T_TILES], f16, tag="rrvw")
                nc.sync.dma_start(out=rrvw[:], in_=rrv_d[w])

                # gather all 8 tiles' sender features for this window
                nf8 = wp.tile([P, T_TILES, 4 * C], f16, tag="nf8")
                nc.gpsimd.dma_gather(
                    out_ap=nf8[:], in_ap=nf_d[:], idxs_ap=sndw[:],
                    num_idxs=EDGE_CAP, num_idxs_reg=EDGE_CAP,
                    elem_size=4 * C)

                pw = pwin.tile([P, 11 * C], f32, tag="pw")

                for half in range(T_TILES // 4):
                    ck = w * (T_TILES // 4) + half
                    hs = half * 4
                    radt = kp.tile([R_DIM, CHUNK], f16, tag="radt")
                    nc.sync.dma_start(out=radt[:], in_=radT_d[ck])
                    h1p = ph.tile([64, CHUNK], f32, tag="ptmp")
                    nc.tensor.matmul(h1p[:], w0sb[:], radt[:],
                                     start=True, stop=True)
                    h1s = kp.tile([64, CHUNK], f16, tag="h1s")
                    nc.scalar.activation(h1s[:], h1p[:], AF.Silu)
                    h2p = ph.tile([64, CHUNK], f32, tag="ptmp")
                    nc.tensor.matmul(h2p[:], w1sb[:], h1s[:],
                                     start=True, stop=True)
                    h2s = kp.tile([64, CHUNK], f16, tag="h2s")
                    nc.scalar.activation(h2s[:], h2p[:], AF.Silu)
                    h3p = ph.tile([64, CHUNK], f32, tag="ptmp")
                    nc.tensor.matmul(h3p[:], w2sb[:], h2s[:],
                                     start=True, stop=True)
                    h3s = kp.tile([64, CHUNK], f16, tag="h3s")
                    nc.scalar.activation(h3s[:], h3p[:], AF.Silu)

                    # gates for the chunk's 4 tiles -> fp16 SBUF
                    mixs = kp.tile([P, 4, 768], f16, tag="mixs")
                    for t4 in range(4):
                        h3sl = h3s[:, t4 * P:(t4 + 1) * P]
                        mix = ph.tile([P, 768], f32, tag="ptmp")
                        nc.tensor.matmul(mix[:, 0:512], h3sl, w3asb[:],
                                         start=True, stop=True)
                        nc.tensor.matmul(mix[:, 512:768], h3sl, w3bsb[:],
                                         start=True, stop=True)
                        nc.scalar.copy(mixs[:, t4, :], mix[:])

                    rn4 = rnall[:, ck * 12:ck * 12 + 12].rearrange(
                        "p (t i) -> p t i", i=3)

                    # i-major views of gathered features
                    ss4 = nf8[:, hs:hs + 4, 0:C]
                    vs4 = nf8[:, hs:hs + 4, C:4 * C].rearrange(
                        "p t (i c) -> p t i c", i=3)
                    rn4c = rn4.unsqueeze(3).to_broadcast([P, 4, 3, C])
                    g0 = mixs[:, :, 0:128]
                    g1 = mixs[:, :, 128:256]
                    g2 = mixs[:, :, 256:384]
                    g3 = mixs[:, :, 384:512]
                    g4 = mixs[:, :, 512:640]
                    g4b = mixs[:, :, 640:768]

                    # t0[c] = sum_j vs[j,c]*rn[j]
                    mscr = tp.tile([P, 4, 3, C], f16, tag="mscr")
                    nc.vector.tensor_tensor(mscr[:], vs4, rn4c,
                                            op=AluOpType.mult)
                    t0a = tp.tile([P, 4, C], f16, tag="t0a")
                    nc.vector.tensor_tensor(t0a[:], mscr[:, :, 0, :],
                                            mscr[:, :, 1, :],
                                            op=AluOpType.add)
                    t04 = tp.tile([P, 4, C], f16, tag="t04")
                    nc.vector.tensor_tensor(t04[:], t0a[:],
                                            mscr[:, :, 2, :],
                                            op=AluOpType.add)

                    # msg layout per tile (1408 cols):
                    #  [ss*g0(128) | t0*g1(128) | vs*g2(3x128 i-major) |
                    #   tp1(3x128) | tp2(3x128)]
                    msg = tp.tile([P, 4, 11 * C], f16, tag="msg")
                    nc.vector.tensor_tensor(msg[:, :, 0:128], ss4, g0,
                                            op=AluOpType.mult)
                    nc.vector.tensor_tensor(msg[:, :, 128:256], t04[:], g1,
                                            op=AluOpType.mult)
                    nc.vector.tensor_tensor(
                        msg[:, :, 256:640].rearrange("p t (i c) -> p t i c",
                                                     i=3),
                        vs4, g2.unsqueeze(2).to_broadcast([P, 4, 3, C]),
                        op=AluOpType.mult)
                    # tp1 = (ss*g3) x rn ; tp2 = (t0*g4) x rn - (vs*g4)/3
                    aa4 = tp.tile([P, 4, C], f16, tag="aa4")
                    nc.vector.tensor_tensor(aa4[:], ss4, g3,
                                            op=AluOpType.mult)
                    dd4 = tp.tile([P, 4, C], f16, tag="dd4")
                    nc.vector.tensor_tensor(dd4[:], t04[:], g4,
                                            op=AluOpType.mult)
                    nc.vector.tensor_tensor(
                        msg[:, :, 640:1024].rearrange("p t (i c) -> p t i c",
                                                      i=3),
                        aa4[:].unsqueeze(2).to_broadcast([P, 4, 3, C]),
                        rn4c, op=AluOpType.mult)
                    nc.vector.tensor_tensor(
                        msg[:, :, 1024:1408].rearrange("p t (i c) -> p t i c",
                                                       i=3),
                        dd4[:].unsqueeze(2).to_broadcast([P, 4, 3, C]),
                        rn4c, op=AluOpType.mult)
                    bb4 = tp.tile([P, 4, 3, C], f16, tag="bb4")
                    nc.vector.tensor_tensor(
                        bb4[:], vs4,
                        g4b.unsqueeze(2).to_broadcast([P, 4, 3, C]),
                        op=AluOpType.mult)
                    nc.vector.tensor_tensor(
                        msg[:, :, 1024:1408],
                        msg[:, :, 1024:1408],
                        bb4[:].rearrange("p t i c -> p t (i c)"),
                        op=AluOpType.add)

                    sel4 = tp.tile([P, 4, P], f16, tag="sel4")
                    nc.vector.tensor_tensor(
                        sel4[:],
                        rrvw[:, hs:hs + 4].unsqueeze(2).to_broadcast(
                            [P, 4, P]),
                        iotasb[:].unsqueeze(1).to_broadcast([P, 4, P]),
                        op=AluOpType.is_equal)

                    for t4 in range(4):
                        tt = hs + t4
                        st = (tt == 0)
                        sp = (tt == T_TILES - 1)
                        sl = sel4[:, t4, :]
                        nc.tensor.matmul(pw[:, 0:512], sl,
                                         msg[:, t4, 0:512],
                                         start=st, stop=sp,
                                         skip_group_check=True)
                        nc.tensor.matmul(pw[:, 512:1024], sl,
                                         msg[:, t4, 512:1024],
                                         start=st, stop=sp,
                                         skip_group_check=True)
                        nc.tensor.matmul(pw[:, 1024:1408], sl,
                                         msg[:, t4, 1024:1408],
                                         start=st, stop=sp,
                                         skip_group_check=True)

                outsb = wp.tile([P, 11 * C], f32, tag="outsb")
                nc.scalar.copy(outsb[:, 0:512], pw[:, 0:512])
                nc.scalar.copy(outsb[:, 512:1024], pw[:, 512:1024])
                nc.scalar.copy(outsb[:, 1024:1408], pw[:, 1024:1408])
                nc.sync.dma_start(out=out_d[w * P:(w + 1) * P, :],
                                  in_=outsb[:])

    nc.finalize()
    return nc


# ---------------------------------------------------------------- entry
def _prepare_weights(w0, w1, w2, w3):
    f16 = np.float16
    w0p = (w0 / np.sqrt(8.0)).astype(f16)
    w1p = (w1 / 8.0).astype(f16)
    w2p = (w2 / 8.0).astype(f16)
    w3p = (w3 / 8.0 / np.sqrt(AVG_NUM_NEIGHBORS)).astype(np.float32).copy()
    w3p[:, 4 * C:5 * C] *= G4_SCALE
    g4 = w3p[:, 512:640]
    w3b = np.concatenate([g4, g4 * (-1.0 / 3.0)], axis=1)
    return (w0p, w1p, w2p, w3p[:, 0:512].astype(f16).copy(),
            w3b.astype(f16).copy())


def _out_col_perm():
    """Device msg v-parts are i-major [i, c]; reference wants c-major
    [c, i].  Returns perm with out_ref[:, j] = out_dev[:, perm[j]]."""
    perm = np.arange(11 * C)
    for b in range(3):  # vs, tp1, tp2 blocks
        base = 2 * C + b * 3 * C
        blk = np.arange(base, base + 3 * C).reshape(3, C)  # dev: [i, c]
        perm[base:base + 3 * C] = blk.T.reshape(-1)        # ref: [c, i]
    return perm


def kernel(vectors, node_feats, radial_embedding, w0, w1, w2, w3, senders,
           receivers):
    global _LAST_IN_MAPS
    _install_ntff_hook()
    from concourse.bass_utils import run_bass_kernel_spmd

    f16 = np.float16
    vectors = np.asarray(vectors, np.float32)
    node_feats = np.asarray(node_feats, np.float32)
    radial = np.asarray(radial_embedding, np.float32)
    senders = np.asarray(senders, np.int32)
    receivers = np.asarray(receivers, np.int32)

    perm, recv_s, cores = _make_windows(senders, receivers, N_NODES)
    W = max(len(c["wins"]) for c in cores)

    key = ("mod", W)
    if key not in _CACHE:
        _CACHE[key] = _build_module(W)
    nc = _CACHE[key]

    w0p, w1p, w2p, w3a, w3b = _prepare_weights(w0, w1, w2, w3)
    iota = np.broadcast_to(np.arange(P, dtype=np.float32),
                           (P, P)).astype(f16).copy()
    # node_feats columns -> [s | vx | vy | vz] (i-major vectors), fp16
    nf_sh = np.empty_like(node_feats)
    nf_sh[:, 0:C] = node_feats[:, 0:C]
    v = node_feats[:, C:].reshape(N_NODES, C, 3)
    for j in range(3):
        nf_sh[:, C + j * C:C + (j + 1) * C] = v[:, :, j]
    nf_f16 = nf_sh.astype(f16)

    in_maps = []
    for c in range(NCORES):
        sndT, rrvT, vecT, radT = _prep_core_arrays(
            W, cores[c]["wins"], perm, recv_s, senders, vectors, radial)
        in_maps.append({
            "node_feats": nf_f16, "w0p": w0p, "w1p": w1p, "w2p": w2p,
            "w3a": w3a, "w3b": w3b, "iota": iota,
            "snd": sndT, "rrv": rrvT, "vec": vecT, "radT": radT,
        })

    _LAST_IN_MAPS = in_maps
    res = run_bass_kernel_spmd(nc, in_maps, core_ids=list(range(NCORES)))

    colp = _out_col_perm()
    out = np.zeros((N_NODES, 11 * C), np.float32)
    for c in range(NCORES):
        co = res.results[c]["out"]
        for w, (ns, nl, _es, _ec) in enumerate(cores[c]["wins"]):
            out[ns:ns + nl] = co[w * P:w * P + nl][:, colp]
    return out


# revision 16
# speedup vs baseline: 1.1367x; 1.1367x over previous
"""MACE MessagePassingConvolution on 8 Trainium2 NeuronCores.

Strategy (graph/data parallel over edges, sorted by receiver):
  - Host: sort edges by receiver, split into 8 contiguous chunks at node
    boundaries.  Within a core, group edges into variable-width node
    "windows" (<=128 nodes and <=T_TILES*128 edges each), padded to a
    static grid of W windows x T_TILES tiles of 128 edges.  node_feats
    columns are pre-shuffled to [s | vx | vy | vz] so vector components are
    i-major on device; the output's vector columns are un-permuted on the
    host afterwards.
  - Device, per 128-edge tile: dma_gather of sender features (fp16),
    radial-MLP gates (PE matmuls, edges on the free dim; the orientation
    flips at the last layer by making h3^T the stationary operand), message
    construction with chunk-batched fp16 vector ops laid out to keep the
    DVE 2x perf mode (long contiguous inner runs), and a one-hot selection
    matmul that scatter-accumulates the 1408-wide fp16 messages into fp32
    PSUM window accumulators.
  - Each window drains PSUM -> SBUF -> DRAM rows at static offsets; the
    host reassembles per-core slabs into the full [N, 1408] output.
"""

import os
import sys

for _p in ("/opt/trn_rl_repo", "/root/.axon_site/_ro/trn_rl_repo"):
    if os.path.isdir(_p) and _p not in sys.path:
        sys.path.insert(0, _p)

import numpy as np

# ---------------------------------------------------------------- constants
C = 128
N_NODES = 10000
N_EDGES = 100000
R_DIM = 8
NCORES = 8
P = 128

T_TILES = 8                 # 128-edge tiles per window (must be mult of 4)
EDGE_CAP = T_TILES * P      # max edges per window
CHUNK = 512                 # edges per radial-MLP chunk (4 tiles)

AVG_NUM_NEIGHBORS = 10.0
SH_L2 = np.sqrt(7.5)
CG_121 = np.sqrt(0.4)
# CG_011*SH_L1 == 1 and CG_110*SH_L1 == 1 exactly, so tp0/tp1 need no
# extra scale; tp2's scale is CG_121*SH_L2 == sqrt(3).
G4_SCALE = CG_121 * SH_L2

_CACHE: dict = {}
_LAST_IN_MAPS = None


def _install_ntff_hook():
    """Make run_bass_kernel_spmd(trace=True) usable under axon: the agent
    image's antenv lacks axon_hooks, so recreate the registry and install
    the ctypes NTFF hook. Harmless if profiling is never requested."""
    import types

    if "antenv.axon_hooks" in sys.modules:
        return
    mod = types.ModuleType("antenv.axon_hooks")
    _h = [None]
    mod.set_axon_ntff_profile_hook = lambda h: _h.__setitem__(0, h)
    mod.get_axon_ntff_profile_hook = lambda: _h[0]
    sys.modules["antenv.axon_hooks"] = mod
    try:
        import antenv

        antenv.axon_hooks = mod
    except ImportError:
        pass
    try:
        from trn_agent_boot.trn_boot import _ntff_profile_via_ctypes

        h = _ntff_profile_via_ctypes("/opt/axon/libaxon_pjrt.so")
        if h is not None:
            mod.set_axon_ntff_profile_hook(h)
    except Exception:
        pass


# ---------------------------------------------------------------- host prep
def _make_windows(senders, receivers, n_nodes):
    """Sort edges by receiver, shard into NCORES chunks at node boundaries,
    then split each core's node range into windows of <=128 nodes and
    <=EDGE_CAP edges."""
    E = senders.shape[0]
    perm = np.argsort(receivers, kind="stable")
    recv_s = receivers[perm]

    splits = [0]
    for c in range(1, NCORES):
        t = (c * E) // NCORES
        while 0 < t < E and recv_s[t] == recv_s[t - 1]:
            t += 1
        splits.append(t)
    splits.append(E)

    deg = np.bincount(receivers, minlength=n_nodes)

    cores = []
    for c in range(NCORES):
        e0, e1 = splits[c], splits[c + 1]
        node_lo = 0 if c == 0 else (int(recv_s[e0]) if e0 < E else n_nodes)
        node_hi = int(recv_s[e1]) if e1 < E else n_nodes
        if c == NCORES - 1:
            node_hi = n_nodes
        if e0 == e1:
            node_lo = node_hi
        wins = []  # (node_start, node_len, edge_start, edge_count)
        n, e = node_lo, e0
        while n < node_hi:
            ns, ecnt = n, 0
            while n < node_hi and n - ns < P:
                d = int(deg[n])
                if ecnt + d > EDGE_CAP:
                    break
                ecnt += d
                n += 1
            assert n > ns, f"node {ns} degree {deg[ns]} exceeds window cap"
            wins.append((ns, n - ns, e, ecnt))
            e += ecnt
        assert e == e1, (c, e, e1)
        cores.append({"wins": wins})
    return perm, recv_s, cores


def _prep_core_arrays(W, wins, perm, recv_s, senders, vectors, radial):
    """Padded per-core device arrays for a static W x T_TILES grid."""
    f16 = np.float16
    L = W * EDGE_CAP
    snd = np.zeros(L, np.int16)
    rrv = np.full(L, -1.0, np.float32)
    vec = np.zeros((L, 3), np.float32)
    vec[:, 0] = 1.0  # pad vectors normalize safely
    rad = np.zeros((L, R_DIM), np.float32)
    for w, (ns, nl, es, ec) in enumerate(wins):
        o = w * EDGE_CAP
        idx = perm[es:es + ec]
        snd[o:o + ec] = senders[idx].astype(np.int16)
        rrv[o:o + ec] = (recv_s[es:es + ec] - ns).astype(np.float32)
        vec[o:o + ec] = vectors[idx]
        rad[o:o + ec] = radial[idx]
    # senders: dma_gather wrapped layout [W, 128, EDGE_CAP//16]
    # idx k of a window lives at [k%16, k//16], replicated to 128 partitions
    sndW = snd.reshape(W, EDGE_CAP // 16, 16).transpose(0, 2, 1)
    sndT = np.tile(sndW, (1, 8, 1)).copy()
    rrvT = rrv.reshape(W, T_TILES, P).transpose(0, 2, 1).astype(f16).copy()
    # vec: all windows in one [128, W*T*3] tensor (tile t of window w at
    # columns (w*T+t)*3 : +3)
    vecT = vec.reshape(W * T_TILES, P, 3).transpose(1, 0, 2).reshape(
        P, W * T_TILES * 3).copy()
    # radial: per 512-edge chunk transposed to [8, 512]
    radT = rad.reshape(L // CHUNK, CHUNK, R_DIM).transpose(0, 2, 1).astype(
        f16).copy()
    return sndT, rrvT, vecT, radT


# ---------------------------------------------------------------- bass build
def _build_module(W):
    import concourse.bass as bass  # noqa: F401
    import concourse.mybir as mybir
    import concourse.tile as tile
    from concourse import bacc
    from concourse.alu_op_type import AluOpType

    f32 = mybir.dt.float32
    f16 = mybir.dt.float16
    i16 = mybir.dt.int16
    AF = mybir.ActivationFunctionType
    X = mybir.AxisListType.X
    NT = W * T_TILES  # total tiles

    nc = bacc.Bacc("TRN2")

    # node_feats columns pre-shuffled on host to [s | vx | vy | vz]
    nf_d = nc.dram_tensor("node_feats", [N_NODES, 4 * C], f16,
                          kind="ExternalInput")
    w0_d = nc.dram_tensor("w0p", [R_DIM, 64], f16, kind="ExternalInput")
    w1_d = nc.dram_tensor("w1p", [64, 64], f16, kind="ExternalInput")
    w2_d = nc.dram_tensor("w2p", [64, 64], f16, kind="ExternalInput")
    w3a_d = nc.dram_tensor("w3a", [64, 512], f16, kind="ExternalInput")
    w3b_d = nc.dram_tensor("w3b", [64, 256], f16, kind="ExternalInput")
    iota_d = nc.dram_tensor("iota", [P, P], f16, kind="ExternalInput")
    snd_d = nc.dram_tensor("snd", [W, P, EDGE_CAP // 16], i16,
                           kind="ExternalInput")
    rrv_d = nc.dram_tensor("rrv", [W, P, T_TILES], f16, kind="ExternalInput")
    vec_d = nc.dram_tensor("vec", [P, NT * 3], f32, kind="ExternalInput")
    radT_d = nc.dram_tensor("radT", [W * EDGE_CAP // CHUNK, R_DIM, CHUNK],
                            f16, kind="ExternalInput")
    out_d = nc.dram_tensor("out", [W * P, 11 * C], f32, kind="ExternalOutput")

    with tile.TileContext(nc) as tc:
        with (
            tc.tile_pool(name="const", bufs=1) as cp,
            tc.tile_pool(name="win", bufs=3) as wp,
            tc.tile_pool(name="chunk", bufs=3) as kp,
            tc.tile_pool(name="tp", bufs=4) as tp,
            tc.tile_pool(name="pwin", bufs=1, space="PSUM") as pwin,
            tc.tile_pool(name="pmix", bufs=2, space="PSUM") as pmix,
            tc.tile_pool(name="ph", bufs=1, space="PSUM") as ph,
        ):
            w0sb = cp.tile([R_DIM, 64], f16)
            nc.sync.dma_start(out=w0sb[:], in_=w0_d[:])
            w1sb = cp.tile([64, 64], f16)
            nc.sync.dma_start(out=w1sb[:], in_=w1_d[:])
            w2sb = cp.tile([64, 64], f16)
            nc.sync.dma_start(out=w2sb[:], in_=w2_d[:])
            w3asb = cp.tile([64, 512], f16)
            nc.sync.dma_start(out=w3asb[:], in_=w3a_d[:])
            w3bsb = cp.tile([64, 256], f16)
            nc.sync.dma_start(out=w3bsb[:], in_=w3b_d[:])
            iotasb = cp.tile([P, P], f16)
            nc.sync.dma_start(out=iotasb[:], in_=iota_d[:])

            # --- batched edge-vector normalization (one Sqrt table load) ---
            vecall = cp.tile([P, NT * 3], f32)
            nc.sync.dma_start(out=vecall[:], in_=vec_d[:])
            sqall = cp.tile([P, NT * 3], f32)
            nc.vector.tensor_tensor(sqall[:], vecall[:], vecall[:],
                                    op=AluOpType.mult)
            s1all = cp.tile([P, NT], f32)
            nc.vector.reduce_sum(
                s1all[:], sqall[:].rearrange("p (t i) -> p t i", i=3), axis=X)
            srall = cp.tile([P, NT], f32)
            nc.scalar.activation(srall[:], s1all[:], AF.Sqrt)
            rinvall = cp.tile([P, NT], f32)
            nc.vector.reciprocal(rinvall[:], srall[:])
            rnall = cp.tile([P, NT * 3], f16)
            nc.vector.tensor_tensor(
                rnall[:].rearrange("p (t i) -> p t i", i=3),
                vecall[:].rearrange("p (t i) -> p t i", i=3),
                rinvall[:].unsqueeze(2).to_broadcast([P, NT, 3]),
                op=AluOpType.mult)

            for w in range(W):
                sndw = wp.tile([P, EDGE_CAP // 16], i16, tag="sndw")
                nc.sync.dma_start(out=sndw[:], in_=snd_d[w])
                rrvw = wp.tile([P, T_TILES], f16, tag="rrvw")
                nc.sync.dma_start(out=rrvw[:], in_=rrv_d[w])

                # gather all 8 tiles' sender features for this window
                nf8 = wp.tile([P, T_TILES, 4 * C], f16, tag="nf8")
                nc.gpsimd.dma_gather(
                    out_ap=nf8[:], in_ap=nf_d[:], idxs_ap=sndw[:],
                    num_idxs=EDGE_CAP, num_idxs_reg=EDGE_CAP,
                    elem_size=4 * C)

                pw = pwin.tile([P, 11 * C], f32, tag="pw")

                for half in range(T_TILES // 4):
                    ck = w * (T_TILES // 4) + half
                    hs = half * 4
                    radt = kp.tile([R_DIM, CHUNK], f16, tag="radt")
                    nc.sync.dma_start(out=radt[:], in_=radT_d[ck])
                    h1p = ph.tile([64, CHUNK], f32, tag="ptmp")
                    nc.tensor.matmul(h1p[:], w0sb[:], radt[:],
                                     start=True, stop=True)
                    h1s = kp.tile([64, CHUNK], f16, tag="h1s")
                    nc.scalar.activation(h1s[:], h1p[:], AF.Silu)
                    h2p = ph.tile([64, CHUNK], f32, tag="ptmp")
                    nc.tensor.matmul(h2p[:], w1sb[:], h1s[:],
                                     start=True, stop=True)
                    h2s = kp.tile([64, CHUNK], f16, tag="h2s")
                    nc.scalar.activation(h2s[:], h2p[:], AF.Silu)
                    h3p = ph.tile([64, CHUNK], f32, tag="ptmp")
                    nc.tensor.matmul(h3p[:], w2sb[:], h2s[:],
                                     start=True, stop=True)
                    h3s = kp.tile([64, CHUNK], f16, tag="h3s")
                    nc.scalar.activation(h3s[:], h3p[:], AF.Silu)

                    # gates for the chunk's 4 tiles -> fp16 SBUF
                    mixs = kp.tile([P, 4, 768], f16, tag="mixs")
                    for t4 in range(4):
                        h3sl = h3s[:, t4 * P:(t4 + 1) * P]
                        mix = pmix.tile([P, 768], f32, tag="mix")
                        nc.tensor.matmul(mix[:, 0:512], h3sl, w3asb[:],
                                         start=True, stop=True)
                        nc.tensor.matmul(mix[:, 512:768], h3sl, w3bsb[:],
                                         start=True, stop=True)
                        nc.scalar.copy(mixs[:, t4, :], mix[:])

                    rn4 = rnall[:, ck * 12:ck * 12 + 12].rearrange(
                        "p (t i) -> p t i", i=3)

                    # i-major views of gathered features
                    ss4 = nf8[:, hs:hs + 4, 0:C]
                    vs4 = nf8[:, hs:hs + 4, C:4 * C].rearrange(
                        "p t (i c) -> p t i c", i=3)
                    rn4c = rn4.unsqueeze(3).to_broadcast([P, 4, 3, C])
                    g0 = mixs[:, :, 0:128]
                    g1 = mixs[:, :, 128:256]
                    g2 = mixs[:, :, 256:384]
                    g3 = mixs[:, :, 384:512]
                    g4 = mixs[:, :, 512:640]
                    g4b = mixs[:, :, 640:768]

                    # t0[c] = sum_j vs[j,c]*rn[j]
                    mscr = tp.tile([P, 4, 3, C], f16, tag="mscr")
                    nc.vector.tensor_tensor(mscr[:], vs4, rn4c,
                                            op=AluOpType.mult)
                    t0a = tp.tile([P, 4, C], f16, tag="t0a")
                    nc.vector.tensor_tensor(t0a[:], mscr[:, :, 0, :],
                                            mscr[:, :, 1, :],
                                            op=AluOpType.add)
                    t04 = tp.tile([P, 4, C], f16, tag="t04")
                    nc.vector.tensor_tensor(t04[:], t0a[:],
                                            mscr[:, :, 2, :],
                                            op=AluOpType.add)

                    # msg layout per tile (1408 cols):
                    #  [ss*g0(128) | t0*g1(128) | vs*g2(3x128 i-major) |
                    #   tp1(3x128) | tp2(3x128)]
                    msg = tp.tile([P, 4, 11 * C], f16, tag="msg")
                    nc.vector.tensor_tensor(msg[:, :, 0:128], ss4, g0,
                                            op=AluOpType.mult)
                    nc.vector.tensor_tensor(msg[:, :, 128:256], t04[:], g1,
                                            op=AluOpType.mult)
                    nc.vector.tensor_tensor(
                        msg[:, :, 256:640].rearrange("p t (i c) -> p t i c",
                                                     i=3),
                        vs4, g2.unsqueeze(2).to_broadcast([P, 4, 3, C]),
                        op=AluOpType.mult)
                    # tp1 = (ss*g3) x rn ; tp2 = (t0*g4) x rn - (vs*g4)/3
                    aa4 = tp.tile([P, 4, C], f16, tag="aa4")
                    nc.vector.tensor_tensor(aa4[:], ss4, g3,
                                            op=AluOpType.mult)
                    dd4 = tp.tile([P, 4, C], f16, tag="dd4")
                    nc.vector.tensor_tensor(dd4[:], t04[:], g4,
                                            op=AluOpType.mult)
                    nc.vector.tensor_tensor(
                        msg[:, :, 640:1024].rearrange("p t (i c) -> p t i c",
                                                      i=3),
                        aa4[:].unsqueeze(2).to_broadcast([P, 4, 3, C]),
                        rn4c, op=AluOpType.mult)
                    nc.vector.tensor_tensor(
                        msg[:, :, 1024:1408].rearrange("p t (i c) -> p t i c",
                                                       i=3),
                        dd4[:].unsqueeze(2).to_broadcast([P, 4, 3, C]),
                        rn4c, op=AluOpType.mult)
                    bb4 = tp.tile([P, 4, 3, C], f16, tag="bb4")
                    nc.vector.tensor_tensor(
                        bb4[:], vs4,
                        g4b.unsqueeze(2).to_broadcast([P, 4, 3, C]),
                        op=AluOpType.mult)
                    nc.vector.tensor_tensor(
                        msg[:, :, 1024:1408],
                        msg[:, :, 1024:1408],
                        bb4[:].rearrange("p t i c -> p t (i c)"),
                        op=AluOpType.add)

                    sel4 = tp.tile([P, 4, P], f16, tag="sel4")
                    nc.vector.tensor_tensor(
                        sel4[:],
                        rrvw[:, hs:hs + 4].unsqueeze(2).to_broadcast(
                            [P, 4, P]),
                        iotasb[:].unsqueeze(1).to_broadcast([P, 4, P]),
                        op=AluOpType.is_equal)

                    for t4 in range(4):
                        tt = hs + t4
                        st = (tt == 0)
                        sp = (tt == T_TILES - 1)
                        sl = sel4[:, t4, :]
                        nc.tensor.matmul(pw[:, 0:512], sl,
                                         msg[:, t4, 0:512],
                                         start=st, stop=sp,
                                         skip_group_check=True)
                        nc.tensor.matmul(pw[:, 512:1024], sl,
                                         msg[:, t4, 512:1024],
                                         start=st, stop=sp,
                                         skip_group_check=True)
                        nc.tensor.matmul(pw[:, 1024:1408], sl,
                                         msg[:, t4, 1024:1408],
                                         start=st, stop=sp,
                                         skip_group_check=True)

                outsb = wp.tile([P, 11 * C], f32, tag="outsb")
                nc.scalar.copy(outsb[:, 0:512], pw[:, 0:512])
                nc.scalar.copy(outsb[:, 512:1024], pw[:, 512:1024])
                nc.scalar.copy(outsb[:, 1024:1408], pw[:, 1024:1408])
                nc.sync.dma_start(out=out_d[w * P:(w + 1) * P, :],
                                  in_=outsb[:])

    nc.finalize()
    return nc


# ---------------------------------------------------------------- entry
def _prepare_weights(w0, w1, w2, w3):
    f16 = np.float16
    w0p = (w0 / np.sqrt(8.0)).astype(f16)
    w1p = (w1 / 8.0).astype(f16)
    w2p = (w2 / 8.0).astype(f16)
    w3p = (w3 / 8.0 / np.sqrt(AVG_NUM_NEIGHBORS)).astype(np.float32).copy()
    w3p[:, 4 * C:5 * C] *= G4_SCALE
    g4 = w3p[:, 512:640]
    w3b = np.concatenate([g4, g4 * (-1.0 / 3.0)], axis=1)
    return (w0p, w1p, w2p, w3p[:, 0:512].astype(f16).copy(),
            w3b.astype(f16).copy())


def _out_col_perm():
    """Device msg v-parts are i-major [i, c]; reference wants c-major
    [c, i].  Returns perm with out_ref[:, j] = out_dev[:, perm[j]]."""
    perm = np.arange(11 * C)
    for b in range(3):  # vs, tp1, tp2 blocks
        base = 2 * C + b * 3 * C
        blk = np.arange(base, base + 3 * C).reshape(3, C)  # dev: [i, c]
        perm[base:base + 3 * C] = blk.T.reshape(-1)        # ref: [c, i]
    return perm


def kernel(vectors, node_feats, radial_embedding, w0, w1, w2, w3, senders,
           receivers):
    global _LAST_IN_MAPS
    _install_ntff_hook()
    from concourse.bass_utils import run_bass_kernel_spmd

    f16 = np.float16
    vectors = np.asarray(vectors, np.float32)
    node_feats = np.asarray(node_feats, np.float32)
    radial = np.asarray(radial_embedding, np.float32)
    senders = np.asarray(senders, np.int32)
    receivers = np.asarray(receivers, np.int32)

    perm, recv_s, cores = _make_windows(senders, receivers, N_NODES)
    W = max(len(c["wins"]) for c in cores)

    key = ("mod", W)
    if key not in _CACHE:
        _CACHE[key] = _build_module(W)
    nc = _CACHE[key]

    w0p, w1p, w2p, w3a, w3b = _prepare_weights(w0, w1, w2, w3)
    iota = np.broadcast_to(np.arange(P, dtype=np.float32),
                           (P, P)).astype(f16).copy()
    # node_feats columns -> [s | vx | vy | vz] (i-major vectors), fp16
    nf_sh = np.empty_like(node_feats)
    nf_sh[:, 0:C] = node_feats[:, 0:C]
    v = node_feats[:, C:].reshape(N_NODES, C, 3)
    for j in range(3):
        nf_sh[:, C + j * C:C + (j + 1) * C] = v[:, :, j]
    nf_f16 = nf_sh.astype(f16)

    in_maps = []
    for c in range(NCORES):
        sndT, rrvT, vecT, radT = _prep_core_arrays(
            W, cores[c]["wins"], perm, recv_s, senders, vectors, radial)
        in_maps.append({
            "node_feats": nf_f16, "w0p": w0p, "w1p": w1p, "w2p": w2p,
            "w3a": w3a, "w3b": w3b, "iota": iota,
            "snd": sndT, "rrv": rrvT, "vec": vecT, "radT": radT,
        })

    _LAST_IN_MAPS = in_maps
    res = run_bass_kernel_spmd(nc, in_maps, core_ids=list(range(NCORES)))

    colp = _out_col_perm()
    out = np.zeros((N_NODES, 11 * C), np.float32)
    for c in range(NCORES):
        co = res.results[c]["out"]
        for w, (ns, nl, _es, _ec) in enumerate(cores[c]["wins"]):
            out[ns:ns + nl] = co[w * P:w * P + nl][:, colp]
    return out
